# revision 43
# baseline (speedup 1.0000x reference)
"""DVAE GNN message-passing kernel for 8 Trainium2 NeuronCores.

Data parallel over batch B=2048 -> 256 graphs/core (2 partition tiles of
128). Each core runs the full 20-step topological scan, weights replicated.

Math (per sample b, step v in 0..19, Hfwd starts at 0):
  gated_u = sigmoid(Wg @ [H_u, e_u] + bg) * (Wm @ [H_u, e_u])
  Hin_v   = sum_u adj[b,u,v] * gated_u      (u >= v rows: constant C_u)
  H_v     = GRUCell(x_v, Hin_v)
  mu,lv   = W1 @ H_19 + b1, W2 @ H_19 + b2

Structure:
  - The adjacency-weighted message sum runs on the TensorEngine as a PSUM
    accumulation group: Hin_v = adjT_masked @ C  (constant part, u >= v)
    plus one matmul diag(adj[:,u,v]) @ G_u per predecessor.  The diagonal
    stationaries are built by tensor_scalar_mul on DVE (native bf16
    identity * per-partition adj scalar); they depend only on static adj,
    so every step's diagonals are enqueued dependency-free.  This keeps the
    per-sample mixing OFF the vector engines (a fused scalar_tensor_tensor
    runs only at 1x there) and keeps the PE warm at 2.4 GHz.
  - Activations stay batch-major [128b, feat]; matmul stationaries are
    PE-transposed activation chunks (bf16 transposes, 1 cyc/row, one PSUM
    bank per round+tile, single copy to SBUF).
  - Software pipelining: each step's tail emits the next step's
    G-independent PE work (in/const/old-diag matmuls and diag builds), and
    closes the next Hin group the moment G_v lands, so the PE FIFO never
    drains across the step seam.  The input DMA is split by first use so
    step 0 starts ~3 us after launch.  Per-step ones-columns live in
    persistent pool buffers preset once at the prologue.
"""

import sys
import numpy as np

for _p in ("/opt/trn_rl_repo",):
    if _p not in sys.path:
        sys.path.insert(0, _p)

B, MAXN, NVT, HS, NZ = 2048, 20, 26, 501, 56
HS2 = HS + 1                  # 502 (even innermost counts for 2x DVE mode)
NVT_EFF = NVT + MAXN          # 46
XDIM = NVT_EFF + 1            # 47
NCORES = 8
BS = B // NCORES              # 256 samples per core
RZ = 2 * HS                   # 1002

# k-chunking of the augmented hidden axis (501 rows + ones row at 501)
CH = [(0, 128), (128, 128), (256, 128), (384, 118)]


def _pack_layout():
    """Column layout (fp32 cells) of the packed static tensor, ordered by
    first use so the DMA can be split.  name -> (row0, nrows, col0, ncols)."""
    ents = {}
    col = 0

    def put(name, row0, nrows, ncols):
        nonlocal col
        ents[name] = (row0, nrows, col, ncols)
        col += ncols

    # DMA ranges ordered by first use: [0:d1] unblocks step 0's Hin
    # (wxnc + pk vertices 0-3); [d1:d2] the step-loop weights; [d2:] the
    # pk tail (vertices 4+, not needed until step 4).
    put("wxnc", 0, 84, HS2)              # rows 0:48 W_in^T+bias, 64:84 C
    put("pk", 0, 84, MAXN * BS)          # rows 0:48 X^T+ones, 64:84 adjT masked
    d1 = HS2 + 4 * BS                    # wxnc + pk vertices 0..3
    for i, (o, s) in enumerate(CH):
        put(f"wrzh{i}", 0, s, 2 * HS2)
    for i, (o, s) in enumerate(CH):
        put(f"whn{i}", 0, s, HS2)
    put("wrzx", 0, XDIM + 1, 2 * HS2)
    d2a = col                            # gate weights end here
    put("adjg0", 0, 128, MAXN * MAXN)
    put("adjg1", 0, 128, MAXN * MAXN)
    for i, (o, s) in enumerate(CH):
        put(f"wg{i}", 0, s, HS2)
    put("wgv", 0, MAXN, HS2)
    for i, (o, s) in enumerate(CH):
        put(f"wm{i}", 0, s, HS2)
    put("wmv", 0, MAXN, HS2)
    put("eye20", 0, MAXN, MAXN)
    for i, (o, s) in enumerate(CH):
        put(f"w12{i}", 0, s, 2 * NZ)
    d2 = HS2 + MAXN * BS                 # end of pk — weights start here
    return ents, col, (d1, d2a, d2)


_PROG = None  # cached Bass program


def _build_program():
    import concourse.bass as bass
    import concourse.tile as tile
    from concourse import bacc, mybir

    f32 = mybir.dt.float32
    f32r = mybir.dt.float32r
    bf16 = mybir.dt.bfloat16
    AF = mybir.ActivationFunctionType
    OP = mybir.AluOpType

    nc = bacc.Bacc("TRN2", target_bir_lowering=False, debug=False)

    ents, ncols, (d1, d2a, d2) = _pack_layout()
    d_wpack = nc.dram_tensor("wpack", [128, ncols], f32r,
                             kind="ExternalInput").ap()
    d_idb = nc.dram_tensor("idb", [128, 128], bf16, kind="ExternalInput").ap()
    d_out = nc.dram_tensor("out", [BS, 2 * NZ], f32, kind="ExternalOutput").ap()

    def mm(out, lhsT, rhs, start, stop):
        nc.tensor.matmul(out, lhsT, rhs, start=start, stop=stop)

    with tile.TileContext(nc) as tc:
        with (
            tc.tile_pool(name="statics", bufs=1) as sp,
            tc.tile_pool(name="gstore", bufs=1) as gsp,
            tc.tile_pool(name="accp", bufs=2) as acp,
            tc.tile_pool(name="diagp", bufs=8) as dgp,
            tc.tile_pool(name="hint", bufs=2) as hip,
            tc.tile_pool(name="work", bufs=1) as wp,
            tc.tile_pool(name="ps", bufs=4, space="PSUM") as pp,
            tc.tile_pool(name="psin", bufs=1, space="PSUM") as ppin,
            tc.tile_pool(name="pshin", bufs=1, space="PSUM") as pphin,
        ):
            WPACK = sp.tile([128, ncols], f32r, tag="wpack", name="wpack")
            IDB = sp.tile([128, 128], bf16, tag="idb", name="idb")
            nc.sync.dma_start(IDB[:, :], d_idb)
            nc.sync.dma_start(WPACK[:, :d1], d_wpack[:, :d1])      # step-0 Hin
            nc.sync.dma_start(WPACK[:, d2:d2a], d_wpack[:, d2:d2a])  # gate w
            nc.sync.dma_start(WPACK[:, d2a:], d_wpack[:, d2a:])    # gated w
            nc.sync.dma_start(WPACK[:, d1:d2], d_wpack[:, d1:d2])  # pk tail

            def sl(name, dt=None):
                r0, nr, c0, ncl = ents[name]
                ap = WPACK[r0:r0 + nr, c0:c0 + ncl]
                return ap.bitcast(dt) if dt else ap

            PK = sl("pk")
            WRZH = [sl(f"wrzh{i}") for i in range(4)]
            WHN = [sl(f"whn{i}") for i in range(4)]
            WRZX = sl("wrzx")
            WXNC = sl("wxnc")
            WG = [sl(f"wg{i}") for i in range(4)]
            WM = [sl(f"wm{i}") for i in range(4)]
            WGV, WMV, EYE = sl("wgv"), sl("wmv"), sl("eye20")
            W12 = [sl(f"w12{i}") for i in range(4)]
            ADJG = [sl(f"adjg{t}", f32) for t in range(2)]

            # persistent state: gated vectors G[u][t], bf16 (moving operand
            # of the diag matmuls AND accumulation source)
            Gs = [[gsp.tile([128, HS2], bf16, tag=f"G{u}_{t}", name=f"G{u}_{t}")
                   for t in range(2)] for u in range(MAXN - 1)]

            HT_final = [None, None]

            # ---- software-pipelined scan: the next step's PE work that does
            # not depend on G_v (diag builds, in/const/old-diag matmuls) is
            # emitted inside step v's tail, so the PE never drains while the
            # slow per-tile GRU tail (gru2 -> th -> gated) resolves. ----

            def emit_diag_builds(v):
                # diagonal stationaries diag(adj[:,u,v]) — static inputs only
                dd = [[None] * MAXN for _ in range(2)]
                for t in range(2):
                    for u in range(v):
                        dg = dgp.tile([128, 128], bf16, tag=f"dg{t}",
                                      name=f"dg{v}_{u}_{t}")
                        sc = ADJG[t][:, u * MAXN + v:u * MAXN + v + 1]
                        nc.vector.tensor_scalar_mul(dg[:, :], IDB[:, :], sc)
                        dd[t][u] = dg
                return dd

            def emit_in(v):
                tiles = [None, None]
                for t in range(2):
                    ip = ppin.tile([128, 512], f32, tag=f"in{t}",
                                   name=f"in{v}_{t}")
                    xsl = PK[0:XDIM + 1, v * BS + t * 128:v * BS + (t + 1) * 128]
                    mm(ip[:, :HS2], xsl, WXNC[0:XDIM + 1, :], start=True,
                       stop=True)
                    tiles[t] = ip
                return tiles

            def emit_hin_head(v, diags):
                # PSUM accumulation group: const part + diag matmuls for all
                # predecessors except u=v-1 (whose G is not ready yet)
                tiles = [None, None]
                for t in range(2):
                    hp = pphin.tile([128, 512], f32, tag=f"hin{t}",
                                    name=f"hin{v}_{t}")
                    mm(hp[:, :HS2],
                       PK[64:84, v * BS + t * 128:v * BS + (t + 1) * 128],
                       WXNC[64:84, :], start=True, stop=(v == 0))
                    for u in range(max(0, v - 1)):
                        mm(hp[:, :HS2], diags[t][u][:, :], Gs[u][t][:, :],
                           start=False, stop=False)
                    tiles[t] = hp
                return tiles

            # one-time ones-column presets: the rotating acc/h buffers keep
            # their col-501 value (copies/gru writes only touch [:, :HS])
            for t in range(2):
                for _b in range(2):
                    a0 = acp.tile([128, HS2], bf16, tag=f"acc{t}",
                                  name=f"accinit{t}_{_b}")
                    nc.gpsimd.memset(a0[:, HS:HS2], 1.0)
                h0 = wp.tile([128, HS2], bf16, tag=f"h{t}", name=f"hinit{t}")
                nc.gpsimd.memset(h0[:, HS:HS2], 1.0)

            cur_diags = emit_diag_builds(0)
            cur_in = emit_in(0)
            cur_hin = emit_hin_head(0, cur_diags)

            for v in range(MAXN):
                inp, acc, hint = cur_in, [None] * 2, [None] * 2
                r_, z_, tmp_, n_ = [None] * 2, [None] * 2, [None] * 2, [None] * 2
                h_, htt, zpp, mpp = [None] * 2, [None] * 2, [None] * 2, [None] * 2

                def stage_consume(t):
                    a = acp.tile([128, HS2], bf16, tag=f"acc{t}",
                                 name=f"acc{v}_{t}")
                    nc.scalar.copy(a[:, :HS], cur_hin[t][:, :HS])
                    acc[t] = a

                def stage_ta(t):
                    a = acc[t]
                    ta = pp.tile([128, 512], bf16, tag="ps", name=f"ta{v}_{t}")
                    for i, (o, wd) in enumerate(CH):
                        nc.tensor.transpose(ta[:wd, i * 128:(i + 1) * 128],
                                            a[:, o:o + wd], IDB[:, :])
                    hi = hip.tile([128, 512], f32r, tag=f"hint{t}",
                                  name=f"hint{v}_{t}")
                    nc.scalar.copy(hi[:, :], ta[:, :])
                    hint[t] = hi

                def stage_gates(t):
                    xsl = PK[0:XDIM + 1, v * BS + t * 128:v * BS + (t + 1) * 128]
                    pr = pp.tile([128, 512], f32, tag="ps", name=f"rzr{v}_{t}")
                    pz = pp.tile([128, 512], f32, tag="ps", name=f"rzz{v}_{t}")
                    ph = pp.tile([128, 512], f32, tag="ps", name=f"hn{v}_{t}")
                    for i, (o, wd) in enumerate(CH):
                        hsl = hint[t][0:wd, i * 128:(i + 1) * 128]
                        mm(pr[:, :HS2], hsl, WRZH[i][:, 0:HS2],
                           start=(i == 0), stop=False)
                        mm(pz[:, :HS2], hsl, WRZH[i][:, HS2:2 * HS2],
                           start=(i == 0), stop=False)
                        mm(ph[:, :HS2], hsl, WHN[i][:, :],
                           start=(i == 0), stop=(i == 3))
                    mm(pr[:, :HS2], xsl, WRZX[:, 0:HS2], start=False, stop=True)
                    mm(pz[:, :HS2], xsl, WRZX[:, HS2:2 * HS2], start=False,
                       stop=True)
                    r_[t], z_[t], tmp_[t] = pr, pz, ph  # raw psum for now

                def stage_gru1(t):
                    pr, pz, ph = r_[t], z_[t], tmp_[t]
                    r = wp.tile([128, HS], f32, tag=f"r{t}", name=f"r{v}_{t}")
                    z = wp.tile([128, HS], f32, tag=f"z{t}", name=f"z{v}_{t}")
                    nc.scalar.activation(r[:, :], pr[:, :HS], AF.Sigmoid)
                    nc.scalar.activation(z[:, :], pz[:, :HS], AF.Sigmoid)
                    tm = wp.tile([128, HS], f32, tag=f"tmp{t}", name=f"tm{v}_{t}")
                    nc.vector.tensor_tensor(tm[:, :], r[:, :], ph[:, :HS],
                                            OP.mult)
                    nc.vector.tensor_tensor(tm[:, :], tm[:, :], inp[t][:, :HS],
                                            OP.add)
                    n = wp.tile([128, HS], f32, tag=f"n{t}", name=f"n{v}_{t}")
                    nc.scalar.activation(n[:, :], tm[:, :], AF.Tanh)
                    r_[t], z_[t], tmp_[t], n_[t] = r, z, tm, n

                def stage_gru2(t):
                    # h = z*(acc - n) + n   (DVE — the GPSIMD chain is 2x
                    # slower and its latency gates the th/gated tail)
                    ge = nc.vector
                    tm, n, z = tmp_[t], n_[t], z_[t]
                    a = acc[t]
                    h = wp.tile([128, HS2], bf16, tag=f"h{t}", name=f"h{v}_{t}")
                    ge.tensor_tensor(tm[:, :], a[:, :HS], n[:, :], OP.subtract)
                    ge.tensor_tensor(tm[:, :], tm[:, :], z[:, :], OP.mult)
                    ge.tensor_tensor(h[:, :HS], tm[:, :], n[:, :], OP.add)
                    h_[t] = h

                def stage_th(t):
                    th = pp.tile([128, 512], bf16, tag="ps", name=f"th{v}_{t}")
                    for i, (o, wd) in enumerate(CH):
                        nc.tensor.transpose(th[:wd, i * 128:(i + 1) * 128],
                                            h_[t][:, o:o + wd], IDB[:, :])
                    ht = hip.tile([128, 512], f32r, tag=f"ht{t}",
                                  name=f"ht{v}_{t}")
                    nc.scalar.copy(ht[:, :], th[:, :])
                    htt[t] = ht
                    if v == MAXN - 1:
                        HT_final[t] = ht

                def stage_gated_mm(t):
                    zp = pp.tile([128, 512], f32, tag="ps", name=f"zp{v}_{t}")
                    mp = pp.tile([128, 512], f32, tag="ps", name=f"mp{v}_{t}")
                    vsel = EYE[:, v:v + 1].broadcast_to([MAXN, 128])
                    for i, (o, wd) in enumerate(CH):
                        hsl = htt[t][0:wd, i * 128:(i + 1) * 128]
                        mm(zp[:, :HS2], hsl, WG[i][:, :], start=(i == 0), stop=False)
                        mm(mp[:, :HS2], hsl, WM[i][:, :], start=(i == 0), stop=False)
                    mm(zp[:, :HS2], vsel, WGV[:, :], start=False, stop=True)
                    mm(mp[:, :HS2], vsel, WMV[:, :], start=False, stop=True)
                    zpp[t], mpp[t] = zp, mp

                def stage_g(t):
                    sg = wp.tile([128, HS2], f32, tag=f"sg{t}", name=f"sg{v}_{t}")
                    nc.scalar.activation(sg[:, :], zpp[t][:, :HS2], AF.Sigmoid)
                    # mp col 501 is 0 -> G col 501 stays 0
                    nc.vector.tensor_tensor(Gs[v][t][:, :], sg[:, :],
                                            mpp[t][:, :HS2], OP.mult)

                # emission order = scheduler priority; stagger the two tiles,
                # and slot the next step's independent PE work (fillers)
                # before the th1/gated1 tail so the PE never drains
                def stage_readout(t):
                    op = pp.tile([128, 512], f32, tag="ps", name=f"op{t}")
                    for i, (o, wd) in enumerate(CH):
                        hsl = HT_final[t][0:wd, i * 128:(i + 1) * 128]
                        mm(op[:, :2 * NZ], hsl, W12[i][:, :],
                           start=(i == 0), stop=(i == 3))
                    ob = wp.tile([128, 2 * NZ], f32, tag=f"ob{t}", name=f"ob{t}")
                    nc.scalar.copy(ob[:, :], op[:, :2 * NZ])
                    nc.sync.dma_start(d_out[t * 128:(t + 1) * 128, :], ob[:, :])

                stage_consume(0); stage_consume(1)
                stage_ta(0); stage_ta(1)
                stage_gates(0); stage_gates(1)
                stage_gru1(0); stage_gru2(0)
                stage_th(0)
                stage_gru1(1); stage_gru2(1)
                if v < MAXN - 1:
                    stage_gated_mm(0)
                    nxt_diags = emit_diag_builds(v + 1)
                    nxt_in = emit_in(v + 1)
                    nxt_hin = emit_hin_head(v + 1, nxt_diags)
                    stage_th(1)
                    stage_gated_mm(1)
                    stage_g(0)
                    # close the next step's Hin group for tile0 as soon as
                    # G_v[0] exists (keeps the PE fed across the step seam)
                    mm(nxt_hin[0][:, :HS2], nxt_diags[0][v][:, :],
                       Gs[v][0][:, :], start=False, stop=True)
                    stage_g(1)
                    mm(nxt_hin[1][:, :HS2], nxt_diags[1][v][:, :],
                       Gs[v][1][:, :], start=False, stop=True)
                    cur_diags, cur_in, cur_hin = nxt_diags, nxt_in, nxt_hin
                else:
                    # overlap tile0's readout with tile1's GRU tail
                    stage_readout(0)
                    stage_th(1)
                    stage_readout(1)

    nc.compile()
    return nc


def _host_prep(types, feats, adj, Wg, bg, Wm, W_ih, b_ih, W_hh, b_hh, W1, b1, W2, b2):
    """Build per-core input maps (numpy only)."""
    f = np.float32
    types = np.asarray(types).astype(np.int64)
    feats = np.asarray(feats, dtype=f)
    adj = np.asarray(adj, dtype=f)
    Wg, bg, Wm = np.asarray(Wg, f), np.asarray(bg, f), np.asarray(Wm, f)
    W_ih, b_ih = np.asarray(W_ih, f), np.asarray(b_ih, f)
    W_hh, b_hh = np.asarray(W_hh, f), np.asarray(b_hh, f)
    W1, b1 = np.asarray(W1, f), np.asarray(b1, f)
    W2, b2 = np.asarray(W2, f), np.asarray(b2, f)

    bsz = types.shape[0]
    bs = bsz // NCORES

    # X^T with ones row: [48, MAXN*bs] per core
    X = np.zeros((bsz, MAXN, XDIM + 1), dtype=f)
    onehot = np.eye(NVT_EFF, dtype=f)[types.reshape(-1) % NVT_EFF]
    X[:, :, :NVT_EFF] = onehot.reshape(bsz, MAXN, NVT_EFF)
    X[:, :, NVT_EFF] = feats
    X[:, :, XDIM] = 1.0

    # constant gated vectors c_u for zero hidden state
    zg = 1.0 / (1.0 + np.exp(-(bg[None, :] + Wg[:, HS:].T)))   # [20, 501]
    C = (zg * Wm[:, HS:].T).astype(f)

    def aug(wT, brow):
        return np.concatenate([wT, brow[None, :]], axis=0).astype(f)

    def pad_rz(a):          # [s, 1002] -> [s, 1004] with per-gate 502 halves
        o = np.zeros((a.shape[0], 2 * HS2), dtype=f)
        o[:, :HS] = a[:, :HS]
        o[:, HS2:HS2 + HS] = a[:, HS:]
        return o

    def pad_h(a):           # [s, 501] -> [s, 502]
        o = np.zeros((a.shape[0], HS2), dtype=f)
        o[:, :HS] = a
        return o

    wrzh = pad_rz(aug(W_hh[:RZ].T, b_hh[:RZ]))
    whn = pad_h(aug(W_hh[RZ:].T, b_hh[RZ:]))
    wrzx = pad_rz(aug(W_ih[:RZ].T, b_ih[:RZ]))
    wxnc = np.zeros((84, HS2), dtype=f)
    wxnc[:XDIM + 1] = pad_h(aug(W_ih[RZ:].T, b_ih[RZ:]))
    wxnc[64:84] = pad_h(C)
    wg = pad_h(np.concatenate([Wg[:, :HS].T, bg[None, :]], axis=0).astype(f))
    wgv = pad_h(np.ascontiguousarray(Wg[:, HS:].T))
    wm = pad_h(np.concatenate([Wm[:, :HS].T, np.zeros((1, HS), f)], axis=0))
    wmv = pad_h(np.ascontiguousarray(Wm[:, HS:].T))
    eye20 = np.eye(MAXN, dtype=f)
    w12 = np.concatenate([np.concatenate([W1.T, W2.T], axis=1),
                          np.concatenate([b1, b2])[None, :]], axis=0).astype(f)
    import ml_dtypes
    identb = np.eye(128, dtype=np.float32).astype(ml_dtypes.bfloat16)

    ents, ncols, _ = _pack_layout()

    def place(pack, name, arr):
        r0, nr, c0, ncl = ents[name]
        assert arr.shape == (nr, ncl), (name, arr.shape, (nr, ncl))
        pack[r0:r0 + nr, c0:c0 + ncl] = arr

    umask = (np.arange(MAXN)[:, None] >= np.arange(MAXN)[None, :]).astype(f)

    in_maps = []
    for c in range(NCORES):
        slc = slice(c * bs, (c + 1) * bs)
        Xc = X[slc]                                   # [bs, 20, 48]
        xt = Xc.transpose(2, 1, 0).reshape(XDIM + 1, MAXN * bs)
        adjc = adj[slc]                               # [bs, 20, 20]
        # adjT[u, v*bs+b] = adj[b,u,v], zeroed where u < v (only u>=v used)
        adjm = adjc.transpose(1, 2, 0) * umask[:, :, None]
        pk = np.zeros((84, MAXN * bs), dtype=f)
        pk[:XDIM + 1] = xt
        pk[64:84] = adjm.reshape(MAXN, MAXN * bs)

        pack = np.zeros((128, ncols), dtype=f)
        place(pack, "pk", pk)
        place(pack, "wxnc", wxnc)
        for i, (o, s) in enumerate(CH):
            place(pack, f"wrzh{i}", wrzh[o:o + s])
            place(pack, f"whn{i}", whn[o:o + s])
            place(pack, f"w12{i}", w12[o:o + s])
            place(pack, f"wg{i}", wg[o:o + s])
            place(pack, f"wm{i}", wm[o:o + s])
        place(pack, "wrzx", wrzx)
        place(pack, "wgv", wgv)
        place(pack, "wmv", wmv)
        place(pack, "eye20", eye20)
        adjg = adjc.reshape(bs, MAXN * MAXN)
        place(pack, "adjg0", adjg[:128])
        place(pack, "adjg1", adjg[128:])
        in_maps.append(dict(wpack=pack, idb=identb))
    return in_maps


def _get_prog():
    global _PROG
    if _PROG is None:
        _PROG = _build_program()
    return _PROG


def kernel(**inputs):
    from concourse.bass_utils import run_bass_kernel_spmd
    nc = _get_prog()
    in_maps = _host_prep(**inputs)
    res = run_bass_kernel_spmd(nc, in_maps, core_ids=list(range(NCORES)))
    out = np.concatenate([r["out"] for r in res.results], axis=0)
    mu = np.ascontiguousarray(out[:, :NZ])
    logvar = np.ascontiguousarray(out[:, NZ:])
    return mu, logvar
